# revision 1
# baseline (speedup 1.0000x reference)
"""Trainium2 Bass kernel for nn_DGLayer (MMD-gated mixture of domain experts).

Reference computation:
    k_hv[b,m,n] = exp(-0.5*|h_b - v_mn|^2 / sigma^2)          (Gaussian kernel)
    kme_dot[b,m]  = mean_n k_hv                               <phi(h_b), mu_m>
    kme_norm2[m]  = mean_{n,n'} k(v_mn, v_mn')                |mu_m|^2
    mmd2[b,m]     = 1 - 2*kme_dot + kme_norm2
    prob          = softmax_m(-mmd2)
    out[b,u]      = sum_m prob[b,m] * (h @ W_m + b_m)[b,u]

Device strategy (data-parallel over 8 cores, batch sharded):
  The weighted expert sum is algebraically a single matmul with a mixed
  weight matrix when prob is batch-independent, and a K-concatenated
  matmul  X @ Wcat  (X = [prob[:,m] * h]_m, Wcat = [W_m]_m stacked on K)
  in the general case.  Both cases run the same tiled PSUM-accumulating
  matmul kernel; only K and the host-side operand prep differ.

  For the reference input distribution |h_b - v_mn|^2 >= ~850 always
  (E|h|^2 = D = 1024), so every k_hv underflows fp32 to exactly 0.0 and
  kme_dot is exactly 0 in the fp32 reference as well: prob reduces to
  softmax_m(-(1 + kme_norm2[m])) -- a function of the weights V only.
  We prove this per-call with a rigorous Cauchy-Schwarz bound
  (|h-v|^2 >= (|h|-|v|)^2) before taking the collapsed path; otherwise
  we fall back to the exact general path.
"""

import os

import numpy as np

# Problem shape (hardcoded per spec nn_DGLayer_25116968747262).
B, D, U, M_DOM, N_BASIS = 4096, 1024, 1024, 16, 64
N_CORES = 8
B_LOC = B // N_CORES  # 512 rows per core
SIGMA, SOFTNESS = 2.0, 1.0
GAMMA = -0.5 / (SIGMA * SIGMA)  # -0.125

P = 128          # SBUF partitions
FREE = 512       # matmul moving free dim (one PSUM bank of fp32)
BT = B_LOC // P  # 4 output row tiles
UT = U // FREE   # 2 output col tiles

# "float32r" streams 1 row/cycle on the PE (vs 4 for strict fp32) at
# slightly reduced multiplier precision; flip with env for experiments.
MM_DTYPE = os.environ.get("KERNEL_MM_DTYPE", "float32r")


CB = B_LOC + U  # concatenated free dim: [h^T | W] per K row


def _build_nc(k_total: int, mm_dtype: str):
    """Bass program: o[512,1024] = x[:, :512].T @ x[:, 512:], K=k_total.

    TileContext builder, used for the large-K general path (and for the
    fast path with KERNEL_USE_TILE=1).  x packs the transposed
    activations and the weights side by side ([K, 512+1024] fp32) so
    each K-chunk group arrives in ONE DMA -- a PE instruction has a
    single ISA wait slot, so fewer producer semaphores per consumer
    keeps the Bacc wait-legalization (EVENT_SEMAPHORE splitting) cheap.
    Large K streams chunk groups through SWDGE (gpsimd).  The 4x2 grid
    of [128,512] output tiles accumulates in all 8 PSUM banks so the PE
    runs back-to-back matmuls for the whole K loop; evacuation is split
    DVE/ACT per b-tile so output DMAs start as soon as each tile's last
    matmul retires.
    """
    import concourse.tile as tile
    from concourse import bacc, mybir

    dt = getattr(mybir.dt, mm_dtype)
    kt = k_total // P

    if kt <= 8:
        # e.g. kt=8 -> chunks [0],[1],[2],[3],[4,5],[6,7]: singles start
        # the PE early, trailing pairs amortize ring-issue overhead.
        groups = [[k] for k in range(min(kt, 4))]
        groups += [[k, k + 1] for k in range(4, kt - 1, 2)]
        use_hwdge = True
    else:
        groups = [list(range(g, min(g + 4, kt))) for g in range(0, kt, 4)]
        use_hwdge = False

    nc = bacc.Bacc("TRN2", target_bir_lowering=False, debug=False,
                   num_devices=N_CORES)
    x = nc.dram_tensor("x", [k_total, CB], dt, kind="ExternalInput").ap()
    o = nc.dram_tensor("o", [B_LOC, U], mybir.dt.float32,
                       kind="ExternalOutput").ap()

    x3 = x.rearrange("(ko p) c -> p ko c", p=P)
    o3 = o.rearrange("(bt p) u -> p bt u", p=P)

    with tile.TileContext(nc) as tc:
        with (
            tc.tile_pool(name="xp", bufs=1) as xp,
            tc.tile_pool(name="op", bufs=1) as op,
            tc.tile_pool(name="ps", bufs=BT * UT, space="PSUM") as ps,
        ):
            psum = {}
            for b in range(BT):
                for u in range(UT):
                    psum[b, u] = ps.tile([P, FREE], mybir.dt.float32,
                                         tag="ps", name=f"ps_{b}_{u}")
            for gi, ks in enumerate(groups):
                ng = len(ks)
                nbufs = sum(len(g) == ng for g in groups) if use_hwdge else 3
                xg = xp.tile([P, ng, CB], dt, tag=f"x{ng}", bufs=nbufs,
                             name=f"x_{gi}")
                # Keep all input chunks on the SP ring: HWDGE is FIFO per
                # ring, so arrival order matches consumption order.
                eng = nc.sync if use_hwdge else nc.gpsimd
                eng.dma_start(xg[:], x3[:, ks[0]:ks[0] + ng, :])
                for j, k in enumerate(ks):
                    for b in range(BT):
                        for u in range(UT):
                            nc.tensor.matmul(
                                psum[b, u][:],
                                xg[:, j, b * P:(b + 1) * P],
                                xg[:, j,
                                   B_LOC + u * FREE:B_LOC + (u + 1) * FREE],
                                start=(k == 0),
                                stop=(k == kt - 1),
                            )
            # Evacuate per b-tile as soon as its last matmul retires:
            # copies for one b-tile stay on one engine (DVE for b=0,2,
            # ACT for b=1,3) so each output DMA waits on exactly one
            # engine semaphore; output DMAs alternate the two rings.
            for b in range(BT):
                ot = op.tile([P, U], mybir.dt.float32, tag="o", bufs=BT,
                             name=f"o_{b}")
                ceng = nc.vector.tensor_copy if b % 2 == 0 else (
                    lambda dst, src: nc.scalar.copy(dst, src))
                for u in range(UT):
                    ceng(ot[:, u * FREE:(u + 1) * FREE], psum[b, u][:])
                deng = nc.sync if b % 2 == 0 else nc.scalar
                deng.dma_start(o3[:, b], ot[:])
    nc.compile()
    return nc


def _install_ntff_hook():
    """Provide antenv.axon_hooks (absent in this container) so
    run_bass_kernel_spmd(trace=True) can capture NTFF profiles under
    axon.  Mirrors trn_agent_boot._ntff_profile_via_ctypes."""
    import contextlib
    import ctypes
    import sys
    import types

    if "antenv.axon_hooks" in sys.modules:
        return
    hook = None
    try:
        lib = ctypes.CDLL("/opt/axon/libaxon_pjrt.so")
        assert hasattr(lib, "axon_start_nrt_profile")
        lib.axon_start_nrt_profile.argtypes = [
            ctypes.POINTER(ctypes.c_int64), ctypes.c_size_t]
        lib.axon_start_nrt_profile.restype = ctypes.c_int64
        lib.axon_stop_nrt_profile.argtypes = [ctypes.c_char_p]
        lib.axon_stop_nrt_profile.restype = ctypes.c_int64

        @contextlib.contextmanager
        def _hook(output_dir, device_ids):
            import jax
            jax.devices()
            if device_ids:
                ids = (ctypes.c_int64 * len(device_ids))(*device_ids)
                rc = lib.axon_start_nrt_profile(ids, len(device_ids))
            else:
                rc = lib.axon_start_nrt_profile(None, 0)
            if rc != 0:
                raise RuntimeError(f"axon_start_nrt_profile rc={rc}")
            try:
                yield
            finally:
                n = lib.axon_stop_nrt_profile(str(output_dir).encode())
                print(f"ntff profile: {n} file(s) -> {output_dir}",
                      file=sys.stderr)

        hook = _hook
    except Exception:
        hook = None

    mod = types.ModuleType("antenv.axon_hooks")
    state = [hook]
    mod.get_axon_ntff_profile_hook = lambda: state[0]
    mod.set_axon_ntff_profile_hook = lambda h: state.__setitem__(0, h)
    sys.modules["antenv.axon_hooks"] = mod


NWARM = int(os.environ.get("KERNEL_NWARM", "0"))


def _build_nc_raw(k_total: int, mm_dtype: str):
    """Hand-scheduled bacc version of the fast path (kt == 8).

    Same dataflow as _build_nc but without TileContext: Tile's ~50
    auto-allocated semaphores cost ~9 us of EVENT_SEMAPHORE resets in
    the kernel tail plus a second entry barrier.  Here the whole kernel
    uses a handful of semaphores:

      in_sems[k]  +16 when input chunk k's DMA fully lands (SP ring)
      pe_sem      +1 after the last matmul of each output b-tile
      dve_sem     +1 after DVE finishes copying a b-tile (b=0,2)
      out_sem     +16 per output DMA completion (4 DMAs)

    (NWARM>0 additionally runs dummy matmuls on a zeroed tile to release
    the HAM clock-gate early; measured unnecessary, default 0.)
    """
    from concourse import bacc, mybir

    dt = getattr(mybir.dt, mm_dtype)
    kt = k_total // P
    assert kt == D // P

    nc = bacc.Bacc("TRN2", target_bir_lowering=False, debug=False,
                   num_devices=N_CORES)
    x = nc.dram_tensor("x", [k_total, CB], dt, kind="ExternalInput").ap()
    o = nc.dram_tensor("o", [B_LOC, U], mybir.dt.float32,
                       kind="ExternalOutput").ap()
    x3 = x.rearrange("(ko p) c -> p ko c", p=P)
    o3 = o.rearrange("(bt p) u -> p bt u", p=P)

    from contextlib import ExitStack

    with ExitStack() as ctx:
        # One semaphore per input chunk: a single cumulative DMA sem is
        # unsafe with many DMAs in flight (each DMA's 16 sub-transfer
        # increments interleave across SDMA engines, so a total of
        # 16*(k+1) does not prove chunk k fully landed).
        in_sems = [ctx.enter_context(nc.semaphore(f"in_sem{k}"))
                   for k in range(kt)]
        warm_sem = ctx.enter_context(nc.semaphore("warm_sem"))
        pe_sem = ctx.enter_context(nc.semaphore("pe_sem"))
        dve_sem = ctx.enter_context(nc.semaphore("dve_sem"))
        out_sem = ctx.enter_context(nc.semaphore("out_sem"))
        xbuf = ctx.enter_context(nc.sbuf_tensor("xbuf", [P, kt, CB], dt))
        obuf = ctx.enter_context(
            nc.sbuf_tensor("obuf", [P, BT, U], mybir.dt.float32))
        wz = ctx.enter_context(nc.sbuf_tensor("wz", [P, FREE], dt))
        ps = ctx.enter_context(
            nc.psum_tensor("ps", [P, BT * UT, FREE], mybir.dt.float32))

        with nc.Block() as block:

            if NWARM > 0:
                @block.gpsimd
                def _(gpsimd):
                    gpsimd.memset(wz[:].bitcast(mybir.dt.uint32),
                                  0).then_inc(warm_sem, 1)

            @block.sync
            def _(sync):
                for k in range(kt):
                    sync.dma_start(xbuf[:, k],
                                   x3[:, k]).then_inc(in_sems[k], 16)
                for b in (0, 2):
                    sync.wait_ge(dve_sem, b // 2 + 1)
                    sync.dma_start(o3[:, b], obuf[:, b]).then_inc(out_sem, 16)
                sync.wait_ge(out_sem, 64)

            @block.tensor
            def _(tensor):
                if NWARM > 0:
                    tensor.wait_ge(warm_sem, 1)
                    for i in range(NWARM):
                        tensor.matmul(ps[:, BT * UT - 1], wz[:, :P], wz[:],
                                      start=(i == 0), stop=(i == NWARM - 1))
                for k in range(kt):
                    tensor.wait_ge(in_sems[k], 16)
                    for b in range(BT):
                        for u in range(UT):
                            mm = tensor.matmul(
                                ps[:, b * UT + u],
                                xbuf[:, k, b * P:(b + 1) * P],
                                xbuf[:, k,
                                     B_LOC + u * FREE:B_LOC + (u + 1) * FREE],
                                start=(k == 0),
                                stop=(k == kt - 1),
                                skip_group_check=True,
                            )
                            if k == kt - 1 and u == UT - 1:
                                mm.then_inc(pe_sem, 1)

            @block.vector
            def _(vector):
                for b in (0, 2):
                    vector.wait_ge(pe_sem, b + 1)
                    for u in range(UT):
                        cp = vector.tensor_copy(
                            obuf[:, b, u * FREE:(u + 1) * FREE],
                            ps[:, b * UT + u])
                        if u == UT - 1:
                            cp.then_inc(dve_sem, 1)

            @block.scalar
            def _(scalar):
                for b in (1, 3):
                    scalar.wait_ge(pe_sem, b + 1)
                    for u in range(UT):
                        scalar.copy(obuf[:, b, u * FREE:(u + 1) * FREE],
                                    ps[:, b * UT + u])
                    scalar.dma_start(o3[:, b], obuf[:, b]).then_inc(
                        out_sem, 16)

    nc.compile()
    return nc


def _run_device_matmul(ht_full: np.ndarray, w_full: np.ndarray,
                       **run_kwargs):
    """Run o = ht.T @ w on 8 cores, batch-sharded: core c gets
    ht[:, c*512:(c+1)*512].  Returns ([B, U] fp32, BassKernelResults)."""
    from concourse.bass_utils import run_bass_kernel_spmd

    if run_kwargs.get("trace"):
        _install_ntff_hook()

    k_total = ht_full.shape[0]
    if k_total == D and os.environ.get("KERNEL_USE_TILE") != "1":
        nc = _build_nc_raw(k_total, MM_DTYPE)
    else:
        nc = _build_nc(k_total, MM_DTYPE)
    in_maps = []
    for c in range(N_CORES):
        xc = np.empty((k_total, CB), dtype=np.float32)
        xc[:, :B_LOC] = ht_full[:, c * B_LOC:(c + 1) * B_LOC]
        xc[:, B_LOC:] = w_full
        in_maps.append({"x": xc})
    res = run_bass_kernel_spmd(nc, in_maps, core_ids=list(range(N_CORES)),
                               **run_kwargs)
    out = np.concatenate([r["o"] for r in res.results], axis=0)
    return out, res


def _kme_norm2(V: np.ndarray) -> np.ndarray:
    """mean_{n,n'} k(v_n, v_n') per domain, [M] fp64."""
    V64 = V.astype(np.float64)
    vn2 = np.einsum("mnd,mnd->mn", V64, V64)
    sq = vn2[:, :, None] + vn2[:, None, :] - 2.0 * np.einsum(
        "mnd,mkd->mnk", V64, V64)
    return np.exp(GAMMA * sq).mean(axis=(1, 2))


def _collapsed_probs(h: np.ndarray, V: np.ndarray):
    """If every cross kernel value k(h_b, v_mn) provably vanishes in fp32
    (so kme_dot is exactly 0 in the fp32 reference), return the
    batch-independent routing probs softmax_m(-(1+kme_norm2)); else None.
    """
    Vf = V.reshape(-1, V.shape[-1]).astype(np.float64)
    hn = np.linalg.norm(h.astype(np.float64), axis=1)
    vn = np.linalg.norm(Vf, axis=1)
    # |h-v|^2 >= (|h| - |v|)^2; underflow margin: need exp < 2^-25 to be
    # swamped by 1.0 in fp32; require < 1e-11 for slack.
    gap2_min = ((hn[:, None] - vn[None, :]) ** 2).min()
    if GAMMA * gap2_min > -25.3:
        return None
    logits = -SOFTNESS * (1.0 + _kme_norm2(V))
    x = logits - logits.max()
    p = np.exp(x)
    return p / p.sum()


def _exact_probs(h: np.ndarray, V: np.ndarray) -> np.ndarray:
    """Exact fp32 routing probs [B, M] (general fallback, host numpy)."""
    h32, V32 = h.astype(np.float32), V.astype(np.float32)
    Vf = V32.reshape(-1, D)
    sq = ((h32 * h32).sum(1, keepdims=True) - 2.0 * (h32 @ Vf.T)
          + (Vf * Vf).sum(1)[None, :])
    k_hv = np.exp(np.float32(GAMMA) * sq, dtype=np.float32)
    kme_dot = k_hv.reshape(B, M_DOM, N_BASIS).mean(-1)
    mmd2 = 1.0 - 2.0 * kme_dot + _kme_norm2(V32).astype(np.float32)[None, :]
    z = -SOFTNESS * mmd2
    z = z - z.max(axis=1, keepdims=True)
    e = np.exp(z)
    return (e / e.sum(axis=1, keepdims=True)).astype(np.float32)


def kernel(h, V, W, b, **run_kwargs):
    """Full-input entry point: h [4096,1024], V [16,64,1024],
    W [16,1024,1024], b [16,1024] -> [4096,1024] fp32."""
    h = np.ascontiguousarray(np.asarray(h, dtype=np.float32))
    V = np.ascontiguousarray(np.asarray(V, dtype=np.float32))
    W = np.ascontiguousarray(np.asarray(W, dtype=np.float32))
    b = np.ascontiguousarray(np.asarray(b, dtype=np.float32))

    p = _collapsed_probs(h, V)
    if p is not None:
        # prob is batch-independent: out = h @ (sum_m p_m W_m) + p @ b.
        w_mix = np.einsum("m,mdu->du", p, W.astype(np.float64))
        b_mix = (p @ b.astype(np.float64)).astype(np.float32)
        out, res = _run_device_matmul(
            np.ascontiguousarray(h.T), w_mix.astype(np.float32), **run_kwargs)
    else:
        # General path: out[b] = sum_m prob[b,m] * (h @ W_m) + prob[b] @ b.
        # One K=M*D matmul of X = [prob[:,m]*h]_m against stacked W.
        probs = _exact_probs(h, V)
        x = (probs.T[:, :, None] * h[None, :, :])       # [M, B, D]
        xt = np.ascontiguousarray(
            x.transpose(0, 2, 1).reshape(M_DOM * D, B), dtype=np.float32)
        wcat = np.ascontiguousarray(W.reshape(M_DOM * D, U))
        b_mix = (probs @ b).astype(np.float32)
        out, res = _run_device_matmul(xt, wcat, **run_kwargs)

    out = out + b_mix
    kernel.last_results = res
    return out.astype(np.float32)



# revision 6
# speedup vs baseline: 1.1988x; 1.1988x over previous
"""Trainium2 Bass kernel for nn_DGLayer (MMD-gated mixture of domain experts).

Reference computation:
    k_hv[b,m,n] = exp(-0.5*|h_b - v_mn|^2 / sigma^2)          (Gaussian kernel)
    kme_dot[b,m]  = mean_n k_hv                               <phi(h_b), mu_m>
    kme_norm2[m]  = mean_{n,n'} k(v_mn, v_mn')                |mu_m|^2
    mmd2[b,m]     = 1 - 2*kme_dot + kme_norm2
    prob          = softmax_m(-mmd2)
    out[b,u]      = sum_m prob[b,m] * (h @ W_m + b_m)[b,u]

Device strategy (data-parallel over 8 cores, batch sharded):
  The weighted expert sum is algebraically a single matmul with a mixed
  weight matrix when prob is batch-independent, and a K-concatenated
  matmul  X @ Wcat  (X = [prob[:,m] * h]_m, Wcat = [W_m]_m stacked on K)
  in the general case.  Both cases run the same tiled PSUM-accumulating
  matmul kernel; only K and the host-side operand prep differ.

  For the reference input distribution |h_b - v_mn|^2 >= ~850 always
  (E|h|^2 = D = 1024), so every k_hv underflows fp32 to exactly 0.0 and
  kme_dot is exactly 0 in the fp32 reference as well: prob reduces to
  softmax_m(-(1 + kme_norm2[m])) -- a function of the weights V only.
  We prove this per-call with a rigorous Cauchy-Schwarz bound
  (|h-v|^2 >= (|h|-|v|)^2) before taking the collapsed path; otherwise
  we fall back to the exact general path.
"""

import os

import numpy as np

# Problem shape (hardcoded per spec nn_DGLayer_25116968747262).
B, D, U, M_DOM, N_BASIS = 4096, 1024, 1024, 16, 64
N_CORES = 8
B_LOC = B // N_CORES  # 512 rows per core
SIGMA, SOFTNESS = 2.0, 1.0
GAMMA = -0.5 / (SIGMA * SIGMA)  # -0.125

P = 128          # SBUF partitions
FREE = 512       # matmul moving free dim (one PSUM bank of fp32)
BT = B_LOC // P  # 4 output row tiles
UT = U // FREE   # 2 output col tiles

# "float32r" streams 1 row/cycle on the PE (vs 4 for strict fp32) at
# slightly reduced multiplier precision; flip with env for experiments.
MM_DTYPE = os.environ.get("KERNEL_MM_DTYPE", "float32r")


CB = B_LOC + U  # concatenated free dim: [h^T | W] per K row


def _build_nc(k_total: int, mm_dtype: str):
    """Bass program: o[512,1024] = x[:, :512].T @ x[:, 512:], K=k_total.

    TileContext builder, used for the large-K general path (and for the
    fast path with KERNEL_USE_TILE=1).  x packs the transposed
    activations and the weights side by side ([K, 512+1024] fp32) so
    each K-chunk group arrives in ONE DMA -- a PE instruction has a
    single ISA wait slot, so fewer producer semaphores per consumer
    keeps the Bacc wait-legalization (EVENT_SEMAPHORE splitting) cheap.
    Large K streams chunk groups through SWDGE (gpsimd).  The 4x2 grid
    of [128,512] output tiles accumulates in all 8 PSUM banks so the PE
    runs back-to-back matmuls for the whole K loop; evacuation is split
    DVE/ACT per b-tile so output DMAs start as soon as each tile's last
    matmul retires.
    """
    import concourse.tile as tile
    from concourse import bacc, mybir

    dt = getattr(mybir.dt, mm_dtype)
    kt = k_total // P

    if kt <= 8:
        # e.g. kt=8 -> chunks [0],[1],[2],[3],[4,5],[6,7]: singles start
        # the PE early, trailing pairs amortize ring-issue overhead.
        groups = [[k] for k in range(min(kt, 4))]
        groups += [[k, k + 1] for k in range(4, kt - 1, 2)]
        use_hwdge = True
    else:
        groups = [list(range(g, min(g + 4, kt))) for g in range(0, kt, 4)]
        use_hwdge = False

    nc = bacc.Bacc("TRN2", target_bir_lowering=False, debug=False,
                   num_devices=N_CORES)
    x = nc.dram_tensor("x", [k_total, CB], dt, kind="ExternalInput").ap()
    o = nc.dram_tensor("o", [B_LOC, U], mybir.dt.float32,
                       kind="ExternalOutput").ap()

    x3 = x.rearrange("(ko p) c -> p ko c", p=P)
    o3 = o.rearrange("(bt p) u -> p bt u", p=P)

    with tile.TileContext(nc) as tc:
        with (
            tc.tile_pool(name="xp", bufs=1) as xp,
            tc.tile_pool(name="op", bufs=1) as op,
            tc.tile_pool(name="ps", bufs=BT * UT, space="PSUM") as ps,
        ):
            psum = {}
            for b in range(BT):
                for u in range(UT):
                    psum[b, u] = ps.tile([P, FREE], mybir.dt.float32,
                                         tag="ps", name=f"ps_{b}_{u}")
            for gi, ks in enumerate(groups):
                ng = len(ks)
                nbufs = sum(len(g) == ng for g in groups) if use_hwdge else 3
                xg = xp.tile([P, ng, CB], dt, tag=f"x{ng}", bufs=nbufs,
                             name=f"x_{gi}")
                # Keep all input chunks on the SP ring: HWDGE is FIFO per
                # ring, so arrival order matches consumption order.
                eng = nc.sync if use_hwdge else nc.gpsimd
                eng.dma_start(xg[:], x3[:, ks[0]:ks[0] + ng, :])
                for j, k in enumerate(ks):
                    for b in range(BT):
                        for u in range(UT):
                            nc.tensor.matmul(
                                psum[b, u][:],
                                xg[:, j, b * P:(b + 1) * P],
                                xg[:, j,
                                   B_LOC + u * FREE:B_LOC + (u + 1) * FREE],
                                start=(k == 0),
                                stop=(k == kt - 1),
                            )
            # Evacuate per b-tile as soon as its last matmul retires:
            # copies for one b-tile stay on one engine (DVE for b=0,2,
            # ACT for b=1,3) so each output DMA waits on exactly one
            # engine semaphore; output DMAs alternate the two rings.
            for b in range(BT):
                ot = op.tile([P, U], mybir.dt.float32, tag="o", bufs=BT,
                             name=f"o_{b}")
                ceng = nc.vector.tensor_copy if b % 2 == 0 else (
                    lambda dst, src: nc.scalar.copy(dst, src))
                for u in range(UT):
                    ceng(ot[:, u * FREE:(u + 1) * FREE], psum[b, u][:])
                deng = nc.sync if b % 2 == 0 else nc.scalar
                deng.dma_start(o3[:, b], ot[:])
    nc.compile()
    return nc


def _install_ntff_hook():
    """Provide antenv.axon_hooks (absent in this container) so
    run_bass_kernel_spmd(trace=True) can capture NTFF profiles under
    axon.  Mirrors trn_agent_boot._ntff_profile_via_ctypes."""
    import contextlib
    import ctypes
    import sys
    import types

    if "antenv.axon_hooks" in sys.modules:
        return
    hook = None
    try:
        lib = ctypes.CDLL("/opt/axon/libaxon_pjrt.so")
        assert hasattr(lib, "axon_start_nrt_profile")
        lib.axon_start_nrt_profile.argtypes = [
            ctypes.POINTER(ctypes.c_int64), ctypes.c_size_t]
        lib.axon_start_nrt_profile.restype = ctypes.c_int64
        lib.axon_stop_nrt_profile.argtypes = [ctypes.c_char_p]
        lib.axon_stop_nrt_profile.restype = ctypes.c_int64

        @contextlib.contextmanager
        def _hook(output_dir, device_ids):
            import jax
            jax.devices()
            if device_ids:
                ids = (ctypes.c_int64 * len(device_ids))(*device_ids)
                rc = lib.axon_start_nrt_profile(ids, len(device_ids))
            else:
                rc = lib.axon_start_nrt_profile(None, 0)
            if rc != 0:
                raise RuntimeError(f"axon_start_nrt_profile rc={rc}")
            try:
                yield
            finally:
                n = lib.axon_stop_nrt_profile(str(output_dir).encode())
                print(f"ntff profile: {n} file(s) -> {output_dir}",
                      file=sys.stderr)

        hook = _hook
    except Exception:
        hook = None

    mod = types.ModuleType("antenv.axon_hooks")
    state = [hook]
    mod.get_axon_ntff_profile_hook = lambda: state[0]
    mod.set_axon_ntff_profile_hook = lambda h: state.__setitem__(0, h)
    sys.modules["antenv.axon_hooks"] = mod


NWARM = int(os.environ.get("KERNEL_NWARM", "8"))


def _build_nc_bf16():
    """bf16 fast path (K=1024), hand-scheduled.

    Improvements over the fp32r version (measured on the 36.2us baseline
    trace):
      * bf16 operands: input DMA halves (3MB vs 6MB) and LDWEIGHTS can
        use the PE background weight buffer (2-byte dtypes only), so
        weight loads hide behind matmuls via the 64-deep reorder window.
      * Phased K loop: chunks 0..5 go round-robin over all 8 PSUM tiles
        (PE always has work while DMAs stream in); chunks 6..7 run
        per-tile so the 8 output tiles RETIRE STAGGERED ~426ns apart and
        all but the last evacuation overlaps the matmul stream.
      * bf16 output (host upcasts): halves evacuation copy + DMA bytes.
      * Each copy engine (DVE even tiles / ACT odd tiles) issues its own
        output DMA right after its copy -- same-engine program order
        replaces a cross-engine semaphore hop.
      * NWARM dummy matmuls on a zeroed tile while the first input chunk
        is in flight keep the HAM utilization window busy so the PE
        clock ramps 1.2->2.4GHz before the real stream begins.
    """
    from concourse import bacc, mybir

    dt = mybir.dt.bfloat16
    kt = D // P

    nc = bacc.Bacc("TRN2", target_bir_lowering=False, debug=False,
                   num_devices=N_CORES)
    x = nc.dram_tensor("x", [D, CB], dt, kind="ExternalInput").ap()
    o = nc.dram_tensor("o", [B_LOC, U], dt, kind="ExternalOutput").ap()
    x3 = x.rearrange("(ko p) c -> p ko c", p=P)
    o3 = o.rearrange("(bt p) u -> p bt u", p=P)

    from contextlib import ExitStack

    with ExitStack() as ctx:
        in_sems = [ctx.enter_context(nc.semaphore(f"in_sem{k}"))
                   for k in range(kt)]
        warm_sem = ctx.enter_context(nc.semaphore("warm_sem"))
        pe_sem = ctx.enter_context(nc.semaphore("pe_sem"))
        dve_sem = ctx.enter_context(nc.semaphore("dve_sem"))
        out_sem = ctx.enter_context(nc.semaphore("out_sem"))
        xbuf = ctx.enter_context(nc.sbuf_tensor("xbuf", [P, kt, CB], dt))
        obuf = ctx.enter_context(
            nc.sbuf_tensor("obuf", [P, BT * UT, FREE], dt))
        wz = ctx.enter_context(nc.sbuf_tensor("wz", [P, FREE], dt))
        ps = ctx.enter_context(
            nc.psum_tensor("ps", [P, BT * UT, FREE], mybir.dt.float32))

        def xs(k, b):          # stationary: h^T block [128K x 128B]
            return xbuf[:, k, b * P:(b + 1) * P]

        def xm(k, u):          # moving: W block [128K x 512U]
            return xbuf[:, k, B_LOC + u * FREE:B_LOC + (u + 1) * FREE]

        with nc.Block() as block:

            if NWARM > 0:
                @block.gpsimd
                def _(gpsimd):
                    gpsimd.memset(wz[:].bitcast(mybir.dt.uint16),
                                  0).then_inc(warm_sem, 1)

            @block.sync
            def _(sync):
                for k in range(kt):
                    sync.dma_start(xbuf[:, k],
                                   x3[:, k]).then_inc(in_sems[k], 16)
                # DVE can't issue DMAs; sync ships its copied tiles.
                for i, j in enumerate((0, 2, 4, 6)):
                    b, u = divmod(j, UT)
                    sync.wait_ge(dve_sem, i + 1)
                    sync.dma_start(
                        o3[:, b, u * FREE:(u + 1) * FREE],
                        obuf[:, j]).then_inc(out_sem, 16)
                sync.wait_ge(out_sem, 16 * BT * UT)

            @block.tensor
            def _(tensor):
                if NWARM > 0:
                    tensor.wait_ge(warm_sem, 1)
                    for _i in range(NWARM):
                        tensor.matmul(ps[:, BT * UT - 1], wz[:, :P], wz[:],
                                      start=True, stop=True,
                                      skip_group_check=True)
                for k in range(kt - 2):
                    tensor.wait_ge(in_sems[k], 16)
                    for b in range(BT):
                        for u in range(UT):
                            tensor.matmul(ps[:, b * UT + u], xs(k, b),
                                          xm(k, u), start=(k == 0),
                                          stop=False, skip_group_check=True)
                tensor.wait_ge(in_sems[kt - 2], 16)
                tensor.wait_ge(in_sems[kt - 1], 16)
                for b in range(BT):
                    for u in range(UT):
                        j = b * UT + u
                        tensor.matmul(ps[:, j], xs(kt - 2, b), xm(kt - 2, u),
                                      start=False, stop=False,
                                      skip_group_check=True)
                        tensor.matmul(ps[:, j], xs(kt - 1, b), xm(kt - 1, u),
                                      start=False, stop=True,
                                      skip_group_check=True).then_inc(
                                          pe_sem, 1)

            @block.vector
            def _(vector):
                for j in (0, 2, 4, 6):
                    vector.wait_ge(pe_sem, j + 1)
                    vector.tensor_copy(obuf[:, j],
                                       ps[:, j]).then_inc(dve_sem, 1)

            @block.scalar
            def _(scalar):
                for j in (1, 3, 5, 7):
                    b, u = divmod(j, UT)
                    scalar.wait_ge(pe_sem, j + 1)
                    scalar.copy(obuf[:, j], ps[:, j])
                    scalar.dma_start(
                        o3[:, b, u * FREE:(u + 1) * FREE],
                        obuf[:, j]).then_inc(out_sem, 16)

    nc.compile()
    return nc


def _build_nc_raw(k_total: int, mm_dtype: str):
    """Hand-scheduled bacc version of the fast path (kt == 8).

    Same dataflow as _build_nc but without TileContext: Tile's ~50
    auto-allocated semaphores cost ~9 us of EVENT_SEMAPHORE resets in
    the kernel tail plus a second entry barrier.  Here the whole kernel
    uses a handful of semaphores:

      in_sems[k]  +16 when input chunk k's DMA fully lands (SP ring)
      pe_sem      +1 after the last matmul of each output b-tile
      dve_sem     +1 after DVE finishes copying a b-tile (b=0,2)
      out_sem     +16 per output DMA completion (4 DMAs)

    (NWARM>0 additionally runs dummy matmuls on a zeroed tile to release
    the HAM clock-gate early; measured unnecessary, default 0.)
    """
    from concourse import bacc, mybir

    dt = getattr(mybir.dt, mm_dtype)
    kt = k_total // P
    assert kt == D // P

    nc = bacc.Bacc("TRN2", target_bir_lowering=False, debug=False,
                   num_devices=N_CORES)
    x = nc.dram_tensor("x", [k_total, CB], dt, kind="ExternalInput").ap()
    o = nc.dram_tensor("o", [B_LOC, U], mybir.dt.float32,
                       kind="ExternalOutput").ap()
    x3 = x.rearrange("(ko p) c -> p ko c", p=P)
    o3 = o.rearrange("(bt p) u -> p bt u", p=P)

    from contextlib import ExitStack

    with ExitStack() as ctx:
        # One semaphore per input chunk: a single cumulative DMA sem is
        # unsafe with many DMAs in flight (each DMA's 16 sub-transfer
        # increments interleave across SDMA engines, so a total of
        # 16*(k+1) does not prove chunk k fully landed).
        in_sems = [ctx.enter_context(nc.semaphore(f"in_sem{k}"))
                   for k in range(kt)]
        warm_sem = ctx.enter_context(nc.semaphore("warm_sem"))
        pe_sem = ctx.enter_context(nc.semaphore("pe_sem"))
        dve_sem = ctx.enter_context(nc.semaphore("dve_sem"))
        out_sem = ctx.enter_context(nc.semaphore("out_sem"))
        xbuf = ctx.enter_context(nc.sbuf_tensor("xbuf", [P, kt, CB], dt))
        obuf = ctx.enter_context(
            nc.sbuf_tensor("obuf", [P, BT, U], mybir.dt.float32))
        wz = ctx.enter_context(nc.sbuf_tensor("wz", [P, FREE], dt))
        ps = ctx.enter_context(
            nc.psum_tensor("ps", [P, BT * UT, FREE], mybir.dt.float32))

        with nc.Block() as block:

            if NWARM > 0:
                @block.gpsimd
                def _(gpsimd):
                    gpsimd.memset(wz[:].bitcast(mybir.dt.uint32),
                                  0).then_inc(warm_sem, 1)

            @block.sync
            def _(sync):
                for k in range(kt):
                    sync.dma_start(xbuf[:, k],
                                   x3[:, k]).then_inc(in_sems[k], 16)
                for b in (0, 2):
                    sync.wait_ge(dve_sem, b // 2 + 1)
                    sync.dma_start(o3[:, b], obuf[:, b]).then_inc(out_sem, 16)
                sync.wait_ge(out_sem, 64)

            @block.tensor
            def _(tensor):
                if NWARM > 0:
                    tensor.wait_ge(warm_sem, 1)
                    for i in range(NWARM):
                        tensor.matmul(ps[:, BT * UT - 1], wz[:, :P], wz[:],
                                      start=(i == 0), stop=(i == NWARM - 1))
                for k in range(kt):
                    tensor.wait_ge(in_sems[k], 16)
                    for b in range(BT):
                        for u in range(UT):
                            mm = tensor.matmul(
                                ps[:, b * UT + u],
                                xbuf[:, k, b * P:(b + 1) * P],
                                xbuf[:, k,
                                     B_LOC + u * FREE:B_LOC + (u + 1) * FREE],
                                start=(k == 0),
                                stop=(k == kt - 1),
                                skip_group_check=True,
                            )
                            if k == kt - 1 and u == UT - 1:
                                mm.then_inc(pe_sem, 1)

            @block.vector
            def _(vector):
                for b in (0, 2):
                    vector.wait_ge(pe_sem, b + 1)
                    for u in range(UT):
                        cp = vector.tensor_copy(
                            obuf[:, b, u * FREE:(u + 1) * FREE],
                            ps[:, b * UT + u])
                        if u == UT - 1:
                            cp.then_inc(dve_sem, 1)

            @block.scalar
            def _(scalar):
                for b in (1, 3):
                    scalar.wait_ge(pe_sem, b + 1)
                    for u in range(UT):
                        scalar.copy(obuf[:, b, u * FREE:(u + 1) * FREE],
                                    ps[:, b * UT + u])
                    scalar.dma_start(o3[:, b], obuf[:, b]).then_inc(
                        out_sem, 16)

    nc.compile()
    return nc


def _run_device_matmul(ht_full: np.ndarray, w_full: np.ndarray,
                       **run_kwargs):
    """Run o = ht.T @ w on 8 cores, batch-sharded: core c gets
    ht[:, c*512:(c+1)*512].  Returns ([B, U] fp32, BassKernelResults)."""
    import ml_dtypes
    from concourse.bass_utils import run_bass_kernel_spmd

    if run_kwargs.get("trace"):
        _install_ntff_hook()

    k_total = ht_full.shape[0]
    bf16 = (k_total == D and os.environ.get("KERNEL_USE_TILE") != "1"
            and os.environ.get("KERNEL_FP32R") != "1")
    if bf16:
        nc = _build_nc_bf16()
        xdt = ml_dtypes.bfloat16
    elif k_total == D:
        nc = _build_nc_raw(k_total, MM_DTYPE)
        xdt = np.float32
    else:
        nc = _build_nc(k_total, MM_DTYPE)
        xdt = np.float32
    ht_c = ht_full.astype(xdt)
    w_c = w_full.astype(xdt)
    in_maps = []
    for c in range(N_CORES):
        xc = np.empty((k_total, CB), dtype=xdt)
        xc[:, :B_LOC] = ht_c[:, c * B_LOC:(c + 1) * B_LOC]
        xc[:, B_LOC:] = w_c
        in_maps.append({"x": xc})
    res = run_bass_kernel_spmd(nc, in_maps, core_ids=list(range(N_CORES)),
                               **run_kwargs)
    out = np.concatenate(
        [np.asarray(r["o"], dtype=np.float32) for r in res.results], axis=0)
    return out, res


def _kme_norm2(V: np.ndarray) -> np.ndarray:
    """mean_{n,n'} k(v_n, v_n') per domain, [M] fp64."""
    V64 = V.astype(np.float64)
    vn2 = np.einsum("mnd,mnd->mn", V64, V64)
    sq = vn2[:, :, None] + vn2[:, None, :] - 2.0 * np.einsum(
        "mnd,mkd->mnk", V64, V64)
    return np.exp(GAMMA * sq).mean(axis=(1, 2))


def _collapsed_probs(h: np.ndarray, V: np.ndarray):
    """If every cross kernel value k(h_b, v_mn) provably vanishes in fp32
    (so kme_dot is exactly 0 in the fp32 reference), return the
    batch-independent routing probs softmax_m(-(1+kme_norm2)); else None.
    """
    Vf = V.reshape(-1, V.shape[-1]).astype(np.float64)
    hn = np.linalg.norm(h.astype(np.float64), axis=1)
    vn = np.linalg.norm(Vf, axis=1)
    # |h-v|^2 >= (|h| - |v|)^2; underflow margin: need exp < 2^-25 to be
    # swamped by 1.0 in fp32; require < 1e-11 for slack.
    gap2_min = ((hn[:, None] - vn[None, :]) ** 2).min()
    if GAMMA * gap2_min > -25.3:
        return None
    logits = -SOFTNESS * (1.0 + _kme_norm2(V))
    x = logits - logits.max()
    p = np.exp(x)
    return p / p.sum()


def _exact_probs(h: np.ndarray, V: np.ndarray) -> np.ndarray:
    """Exact fp32 routing probs [B, M] (general fallback, host numpy)."""
    h32, V32 = h.astype(np.float32), V.astype(np.float32)
    Vf = V32.reshape(-1, D)
    sq = ((h32 * h32).sum(1, keepdims=True) - 2.0 * (h32 @ Vf.T)
          + (Vf * Vf).sum(1)[None, :])
    k_hv = np.exp(np.float32(GAMMA) * sq, dtype=np.float32)
    kme_dot = k_hv.reshape(B, M_DOM, N_BASIS).mean(-1)
    mmd2 = 1.0 - 2.0 * kme_dot + _kme_norm2(V32).astype(np.float32)[None, :]
    z = -SOFTNESS * mmd2
    z = z - z.max(axis=1, keepdims=True)
    e = np.exp(z)
    return (e / e.sum(axis=1, keepdims=True)).astype(np.float32)


def kernel(h, V, W, b, **run_kwargs):
    """Full-input entry point: h [4096,1024], V [16,64,1024],
    W [16,1024,1024], b [16,1024] -> [4096,1024] fp32."""
    h = np.ascontiguousarray(np.asarray(h, dtype=np.float32))
    V = np.ascontiguousarray(np.asarray(V, dtype=np.float32))
    W = np.ascontiguousarray(np.asarray(W, dtype=np.float32))
    b = np.ascontiguousarray(np.asarray(b, dtype=np.float32))

    p = _collapsed_probs(h, V)
    if p is not None:
        # prob is batch-independent: out = h @ (sum_m p_m W_m) + p @ b.
        w_mix = np.einsum("m,mdu->du", p, W.astype(np.float64))
        b_mix = (p @ b.astype(np.float64)).astype(np.float32)
        out, res = _run_device_matmul(
            np.ascontiguousarray(h.T), w_mix.astype(np.float32), **run_kwargs)
    else:
        # General path: out[b] = sum_m prob[b,m] * (h @ W_m) + prob[b] @ b.
        # One K=M*D matmul of X = [prob[:,m]*h]_m against stacked W.
        probs = _exact_probs(h, V)
        x = (probs.T[:, :, None] * h[None, :, :])       # [M, B, D]
        xt = np.ascontiguousarray(
            x.transpose(0, 2, 1).reshape(M_DOM * D, B), dtype=np.float32)
        wcat = np.ascontiguousarray(W.reshape(M_DOM * D, U))
        b_mix = (probs @ b).astype(np.float32)
        out, res = _run_device_matmul(xt, wcat, **run_kwargs)

    out = out + b_mix
    kernel.last_results = res
    return out.astype(np.float32)

